# revision 10
# baseline (speedup 1.0000x reference)
"""Pairwise cosine-similarity kernel for Trainium2 (8 NeuronCores, SPMD).

Computes out = 16 * normalize(x1) @ normalize(x2).T for x1, x2 [8192, 512] f32.

Sharding: x1 rows are split across the 8 cores (1024 rows each); x2 is
replicated. Each core computes its [1024, 8192] slice of the output.

Host-side prep is layout/dtype only: bf16 casts, pre-transposed copies of x1
and x2 (no on-device PE transposes), and an fp8-e4m3 copy of x2 used only as
the norm source. The output travels as fp16 and is upcast to f32 on host.
All FLOPs (norms, normalization, GEMM, scaling) run on device.

Per-core schedule:
  - x1 norms: DVE square-accumulate -> ACT sqrt -> DVE clamp/recip ->
    inv1 = 16/n1 [128, 8] f32 (column mt = scale for m-tile mt).
  - Per column group (cg = 1024 x2 columns): norms of x2 rows q*8+c sit at
    inv2[q, c], so inv2's [128, 8] bf16 memory layout IS the flattened
    column-scale vector: one tiny SBUF->SBUF DMA lowers it to [1, 1024] and
    gpsimd.partition_broadcast replicates it to bc [128, 1024]. No PE
    broadcast matmuls, no diag build. Stats (ACT Square + DVE/ACT row-sum
    accums) run two cgs ahead of the GEMM so chain latency never gates PE.
  - GEMM per (cg, m-tile): two psum [128, 512] over 4 K-chunks (bf16), psum
    pool = all 8 banks (4 m-tiles of runway). Epilogue folds both
    normalizations into the PSUM->SBUF copy:
      j0: DVE scalar_tensor_tensor   out = (psum * inv1) * bc
      j1: ACT Copy(scale=inv1), with x2T's j1 columns pre-scaled by bc on
          GpSimd one cg ahead.
    Outputs collect in [128, 2, 1024] fp16 m-pair tiles, DMA'd from gpsimd.
  - cg0 runs k-major in 4-m-tile groups so the PE starts on the first
    K-chunks as they land; cg0/cg1 use row-scale-only copies plus a
    second-pass column scale (their bc would otherwise gate the pipeline).
"""

import sys

for _p in ("/root/.axon_site/_ro/trn_rl_repo", "/opt/trn_rl_repo"):
    if _p not in sys.path:
        sys.path.append(_p)

import ml_dtypes
import numpy as np

import concourse.bass as bass
import concourse.tile as tile
from concourse import bacc, mybir
from concourse.bass_utils import run_bass_kernel_spmd

F32 = mybir.dt.float32
BF16 = mybir.dt.bfloat16
FP16 = mybir.dt.float16
FP8 = mybir.dt.float8e4
P = 128
SCALE = 16.0
EPS = 1e-8

N_CORES = 8
N1 = 8192  # x1 rows (total)
N2 = 8192  # x2 rows
D = 512  # feature dim
KC = D // P  # 4 K-chunks
CGW = 1024  # column-group width
N_CGS = N2 // CGW  # 8
MT_N = (N1 // N_CORES) // P  # 8 m-tiles per core

MUL = mybir.AluOpType.mult
ADD = mybir.AluOpType.add
ACTF = mybir.ActivationFunctionType

_PROGRAM_CACHE = {}


def build_program():
    n1l = N1 // N_CORES  # 1024 local x1 rows

    nc = bacc.Bacc("TRN2", target_bir_lowering=False, debug=False,
                   num_devices=N_CORES)
    x1n = nc.dram_tensor("x1n", [n1l, D], BF16, kind="ExternalInput")
    x1t = nc.dram_tensor("x1t", [D, n1l], BF16, kind="ExternalInput")
    x2t = nc.dram_tensor("x2t", [D, N2], BF16, kind="ExternalInput")
    x2n8 = nc.dram_tensor("x2n8", [N2, D], FP8, kind="ExternalInput")
    out = nc.dram_tensor("out", [n1l, N2], FP16, kind="ExternalOutput")

    with tile.TileContext(nc) as tc:
        with (
            tc.tile_pool(name="xt", bufs=1) as xt,
            tc.tile_pool(name="sq", bufs=3) as sqp,
            tc.tile_pool(name="stat", bufs=4) as stat,
            tc.tile_pool(name="bcf", bufs=4) as bcfp,
            tc.tile_pool(name="bc", bufs=4) as bcp,
            tc.tile_pool(name="outp", bufs=6) as outp,
            tc.tile_pool(name="ps", bufs=8, space="PSUM") as psp,
        ):
            # DRAM access patterns
            x1n_r = x1n.ap().rearrange("(h mt p) e -> h p mt e",
                                       h=2, mt=MT_N // 2, p=P)
            x1t_r = x1t.ap().rearrange("(k p) n -> p k n", k=KC, p=P)
            x2t_r = x2t.ap().rearrange(
                "(k p) (cg n) -> cg p k n", k=KC, p=P, cg=N_CGS, n=CGW
            )
            # stats row grouping: group column c holds x2 rows cg*1024+q*8+c
            # so inv2's [128, 8] layout is the flat per-column scale vector.
            x2n_r = x2n8.ap().rearrange(
                "(cg q c) e -> cg q c e", cg=N_CGS, q=P, c=CGW // P
            )
            out_r = out.ap().rearrange(
                "(mp mi p) (cg n) -> cg mp p mi n", mi=2, p=P,
                cg=N_CGS, n=CGW
            )

            # Persistent SBUF tiles
            x1T = xt.tile([P, KC, n1l], BF16, name="x1T")
            x2T = [xt.tile([P, KC, CGW], BF16, tag=f"x2T_{cg}",
                           name=f"x2T_{cg}") for cg in range(N_CGS)]
            x1ld = xt.tile([P, MT_N, D], BF16, name="x1ld")
            x2ld = [xt.tile([P, CGW // P, D], FP8, tag=f"x2ld_{cg}",
                            name=f"x2ld_{cg}") for cg in range(N_CGS)]
            inv1 = xt.tile([P, MT_N], F32, name="inv1")
            bc_t = [bcp.tile([P, CGW], BF16, tag="bc", name=f"bc_{cg}")
                    for cg in range(N_CGS)]

            # ---- input DMAs up front (SP queue), priority order ----------
            nc.sync.dma_start(x1T[:, 0], x1t_r[:, 0])
            nc.sync.dma_start(x2T[0][:, 0], x2t_r[0][:, 0])
            nc.sync.dma_start(x1ld[:, 0 : MT_N // 2], x1n_r[0])
            nc.sync.dma_start(x1ld[:, MT_N // 2 : MT_N], x1n_r[1])
            nc.sync.dma_start(x2ld[0][:], x2n_r[0])
            for k in range(1, KC):
                nc.sync.dma_start(x1T[:, k], x1t_r[:, k])
                nc.sync.dma_start(x2T[0][:, k], x2t_r[0][:, k])
            for cg in range(1, N_CGS):
                nc.sync.dma_start(x2ld[cg][:], x2n_r[cg])
                nc.sync.dma_start(x2T[cg][:], x2t_r[cg])

            def finish_stats(ssq, inv_dst, scale_const, tagp):
                """inv_dst = scale / max(sqrt(ssq), EPS)."""
                nrm = stat.tile([P, ssq.shape[1]], F32, tag=f"{tagp}_nrm")
                nc.scalar.activation(nrm[:], ssq[:], ACTF.Sqrt)
                nc.vector.tensor_scalar_max(nrm[:], nrm[:], EPS)
                with nc.allow_low_precision(
                    reason="bf16 column scales add ~2e-3 rel err, "
                    "well inside the 2e-2 budget"
                ):
                    nc.vector.reciprocal(inv_dst, nrm[:])
                if scale_const != 1.0:
                    nc.vector.tensor_scalar_mul(inv_dst, inv_dst, scale_const)

            # ---- x1 stats: DVE square-accumulate (early, off ACT) --------
            ssq1 = stat.tile([P, MT_N], F32, tag="x1_ssq")
            for mt in range(MT_N):
                dum = sqp.tile([P, D], BF16, tag="x1dum")
                nc.vector.scalar_tensor_tensor(
                    dum[:], x1ld[:, mt], 1.0, x1ld[:, mt],
                    op0=MUL, op1=MUL,
                    accum_out=ssq1[:, mt : mt + 1],
                )
            finish_stats(ssq1, inv1[:], SCALE, "x1")

            def prep_stats(cg):
                """x2 norms for the cg: ACT squares, accums 6 DVE / 2 ACT."""
                inv2 = stat.tile([P, CGW // P], BF16, tag="inv2",
                                 name=f"inv2_{cg}")
                ssq = stat.tile([P, CGW // P], F32, tag="x2_ssq",
                                name=f"x2ssq_{cg}")
                for h in range(2):
                    sq_t = sqp.tile([P, 4, D], BF16, tag="x2sq")
                    nc.scalar.activation(
                        sq_t[:], x2ld[cg][:, h * 4 : (h + 1) * 4],
                        ACTF.Square,
                    )
                    for j in range(4):
                        acc = ssq[:, h * 4 + j : h * 4 + j + 1]
                        if j == 3:
                            dumc = sqp.tile([P, D], BF16, tag="x2dum")
                            nc.scalar.activation(
                                dumc[:], sq_t[:, j], ACTF.Copy,
                                accum_out=acc,
                            )
                        else:
                            nc.vector.tensor_scalar(
                                sq_t[:, j], sq_t[:, j], 1.0, 0.0, MUL, ADD,
                                accum_out=acc,
                            )
                finish_stats(ssq, inv2[:], 1.0, "x2")
                return inv2

            def prep_bc(cg, inv2, prescale):
                """bc_t[cg] = partition-broadcast of the flattened inv2."""
                bcf = bcfp.tile([1, CGW], BF16, tag="bcf", name=f"bcf_{cg}")
                nc.gpsimd.dma_start(bcf[:], inv2[:])
                nc.gpsimd.partition_broadcast(bc_t[cg][:], bcf[:], channels=P)
                if prescale:
                    # scale the j1 half of the transposed operand (GpSimd)
                    for k in range(KC):
                        nc.gpsimd.tensor_mul(
                            x2T[cg][:, k, 512:CGW], x2T[cg][:, k, 512:CGW],
                            bc_t[cg][:, 512:CGW],
                        )

            def out_tile(cg, mp):
                return outp.tile([P, 2, CGW], FP16, tag="ot",
                                 name=f"ot_{cg}_{mp}")

            def epilogue(cg, mt, ps0, ps1, ot, fold_bc):
                sl = ot[:, mt % 2]
                if fold_bc:
                    nc.vector.scalar_tensor_tensor(
                        sl[:, 0:512], ps0[:], inv1[:, mt : mt + 1],
                        bc_t[cg][:, 0:512], op0=MUL, op1=MUL,
                    )
                else:
                    nc.vector.tensor_scalar_mul(
                        sl[:, 0:512], ps0[:], inv1[:, mt : mt + 1]
                    )
                nc.scalar.activation(
                    sl[:, 512:CGW], ps1[:], ACTF.Copy,
                    scale=inv1[:, mt : mt + 1],
                )

            def gemm_m(cg, mt, ot, mode):
                """m-major GEMM + epilogue for one (cg, m-tile)."""
                ps0 = psp.tile([P, 512], F32, tag="ps", name=f"ps0_{cg}_{mt}")
                ps1 = psp.tile([P, 512], F32, tag="ps", name=f"ps1_{cg}_{mt}")
                for k in range(KC):
                    nc.tensor.matmul(
                        ps0[:], lhsT=x1T[:, k, mt * P : (mt + 1) * P],
                        rhs=x2T[cg][:, k, 0:512],
                        start=(k == 0), stop=(k == KC - 1),
                    )
                    nc.tensor.matmul(
                        ps1[:], lhsT=x1T[:, k, mt * P : (mt + 1) * P],
                        rhs=x2T[cg][:, k, 512:CGW],
                        start=(k == 0), stop=(k == KC - 1),
                    )
                epilogue(cg, mt, ps0, ps1, ot, fold_bc=(mode == "pre"))
                if mode == "pre" and mt % 2 == 1:
                    nc.sync.dma_start(out_r[cg, mt // 2], ot[:])

            def gemm_cg0_group(mts, ots):
                """cg0 startup: k-major over 4 m-tiles (8 psum banks)."""
                pss = {}
                for mt in mts:
                    pss[mt] = (
                        psp.tile([P, 512], F32, tag="ps", name=f"c0ps0_{mt}"),
                        psp.tile([P, 512], F32, tag="ps", name=f"c0ps1_{mt}"),
                    )
                for k in range(KC):
                    for mt in mts:
                        nc.tensor.matmul(
                            pss[mt][0][:],
                            lhsT=x1T[:, k, mt * P : (mt + 1) * P],
                            rhs=x2T[0][:, k, 0:512],
                            start=(k == 0), stop=(k == KC - 1),
                        )
                        nc.tensor.matmul(
                            pss[mt][1][:],
                            lhsT=x1T[:, k, mt * P : (mt + 1) * P],
                            rhs=x2T[0][:, k, 512:CGW],
                            start=(k == 0), stop=(k == KC - 1),
                        )
                for mt in mts:
                    epilogue(0, mt, pss[mt][0], pss[mt][1], ots[mt // 2],
                             fold_bc=False)

            def second_pass(cg, ots):
                """post-mode: column-scale whole m-pairs by bc, then DMA."""
                for mp, ot in ots.items():
                    nc.vector.tensor_mul(
                        ot[:], ot[:],
                        bc_t[cg][:, None, :].to_broadcast((P, 2, CGW)),
                    )
                    nc.sync.dma_start(out_r[cg, mp], ot[:])

            # ---- emission schedule --------------------------------------
            ots0 = {mp: out_tile(0, mp) for mp in range(4)}
            gemm_cg0_group([0, 1, 2, 3], ots0)
            inv2_0 = prep_stats(0)
            gemm_cg0_group([4, 5, 6, 7], ots0)
            inv2_1 = prep_stats(1)
            prep_bc(0, inv2_0, prescale=False)
            second_pass(0, ots0)
            prep_bc(1, inv2_1, prescale=False)
            inv2_by_cg = {2: prep_stats(2)}
            prep_bc(2, inv2_by_cg[2], prescale=True)

            for cg in range(1, N_CGS):
                mode = "post" if cg == 1 else "pre"
                ots = {mp: out_tile(cg, mp) for mp in range(4)}
                for mt in range(0, 4):
                    gemm_m(cg, mt, ots[mt // 2], mode)
                if cg + 2 < N_CGS:
                    inv2_by_cg[cg + 2] = prep_stats(cg + 2)
                    prep_bc(cg + 2, inv2_by_cg[cg + 2], prescale=True)
                for mt in range(4, MT_N):
                    gemm_m(cg, mt, ots[mt // 2], mode)
                if mode == "post":
                    second_pass(cg, ots)

    nc.compile()
    return nc


def _get_program():
    key = "default"
    if key not in _PROGRAM_CACHE:
        _PROGRAM_CACHE[key] = build_program()
    return _PROGRAM_CACHE[key]


def make_in_maps(x1: np.ndarray, x2: np.ndarray) -> list:
    x1 = np.asarray(x1, dtype=np.float32)
    x2 = np.asarray(x2, dtype=np.float32)
    assert x1.shape == (N1, D) and x2.shape == (N2, D), (x1.shape, x2.shape)
    fp8_np = mybir.dt.np(FP8)
    x1_b = x1.astype(ml_dtypes.bfloat16)
    x2_b = x2.astype(ml_dtypes.bfloat16)
    x2t_b = np.ascontiguousarray(x2_b.T)
    x2n8 = x2.astype(fp8_np)
    rows = N1 // N_CORES
    maps = []
    for c in range(N_CORES):
        sl = x1_b[c * rows : (c + 1) * rows]
        maps.append({
            "x1n": np.ascontiguousarray(sl),
            "x1t": np.ascontiguousarray(sl.T),
            "x2t": x2t_b,
            "x2n8": x2n8,
        })
    return maps


def kernel(x1: np.ndarray, x2: np.ndarray) -> np.ndarray:
    nc = _get_program()
    in_maps = make_in_maps(x1, x2)
    res = run_bass_kernel_spmd(nc, in_maps, core_ids=list(range(N_CORES)))
    return np.concatenate(
        [res.results[c]["out"] for c in range(N_CORES)], axis=0
    ).astype(np.float32)


if __name__ == "__main__":
    rng = np.random.default_rng(0)
    a = rng.standard_normal((N1, D), dtype=np.float32)
    b = rng.standard_normal((N2, D), dtype=np.float32)
    got = kernel(a, b)
    n1 = np.maximum(np.linalg.norm(a, axis=-1, keepdims=True), EPS)
    n2 = np.maximum(np.linalg.norm(b, axis=-1, keepdims=True), EPS)
    want = SCALE * (a / n1) @ (b / n2).T
    err = np.abs(got - want)
    rel = np.linalg.norm(got - want) / np.linalg.norm(want)
    print(f"max abs err: {err.max():.3e}  rel: {rel:.3e}")
